# revision 1
# baseline (speedup 1.0000x reference)
"""CrossAttention Trainium2 kernel.

Problem (hardcoded): B=8, T=256, S=4096, E=512, KV=768, H=8, D=64.
Sharding: data-parallel over B — one batch per NeuronCore (8 cores).

Per-core dataflow (one batch, all layouts staged host-side):
  inputs (bf16 unless noted):
    ctxT  [768, 4096]   = context[b].T          (KV on partitions)
    xT    [512, 256]    = x[b].T
    m01   [128, 32] f32 = 1.0 where key kept, 0.0 where masked (s=sc*128+p)
    wqT   [512, 512]    = Wq.T * D^-0.5  (scale folded, exact pow2)
    wkvT  [768, 1024]   = Wkv.T
    woT   [512, 512]    = Wo.T
    bo_r  [128, 4] f32  = bo.reshape(4,128).T
  device:
    QT    = wqT.T @ xT            -> [512c, 256t]   (c-major, 4 chunks)
    KT    = wkvT[:, :512].T @ ctxT -> [512c, 4096s] (c-major, 4 chunks = head pairs)
    V'    = ctxT.T @ wkvT[:, 512:] -> [4096s, 8h*65] (64 vals + ones col per head),
            rows multiplied by m01 (mask folded into V' => no -inf anywhere)
    scoresT[s,t] per head = KT_h slices as lhsT, QT_h as rhs (K=64, head pair
            packed into PE row groups 0:64 / 64:128)
    expsT = Exp(scoresT)  (no max subtraction needed: |scores| <~ 8)
    PV    = V'_h-as-lhsT @ expsT -> [65, 256] psum; row 64 = softmax denom
    norm  = reciprocal(denom) broadcast via K=1 fp32 matmul; OT = PV * recip
    outT  = woT.T @ OT + bo -> [512e, 256t] -> host transposes back.

ctx DMA is quartered along S and kv-proj consumption follows arrival order.
Scores for 4 s-chunks of one head land in one [128,1024] psum tile so a
single ACTIVATE(Exp) covers them (ACT op overhead would otherwise bind).
"""

import sys

sys.path.insert(0, "/opt/trn_rl_repo")

import numpy as np
import ml_dtypes
from contextlib import ExitStack

import concourse.bass as bass
import concourse.bacc as bacc
import concourse.tile as tile
from concourse import mybir
from concourse import bass_utils

BF16 = mybir.dt.bfloat16
F32 = mybir.dt.float32
NPBF16 = ml_dtypes.bfloat16

B, T, S, E, KV, H, D = 8, 256, 4096, 512, 768, 8, 64
NC_CORES = 8


def _build_program():
    nc = bacc.Bacc("TRN2", target_bir_lowering=False, debug=False)

    ctxT_d = nc.dram_tensor("ctxT", [KV, S], BF16, kind="ExternalInput").ap()
    xT_d = nc.dram_tensor("xT", [E, T], BF16, kind="ExternalInput").ap()
    m01_d = nc.dram_tensor("m01", [128, 32], F32, kind="ExternalInput").ap()
    wqT_d = nc.dram_tensor("wqT", [E, 512], BF16, kind="ExternalInput").ap()
    wkvT_d = nc.dram_tensor("wkvT", [KV, 1024], BF16, kind="ExternalInput").ap()
    woT_d = nc.dram_tensor("woT", [512, E], BF16, kind="ExternalInput").ap()
    bo_d = nc.dram_tensor("bo_r", [128, 4], F32, kind="ExternalInput").ap()
    outT_d = nc.dram_tensor("outT", [4, 128, T], F32, kind="ExternalOutput").ap()

    ctxT_v = ctxT_d.rearrange("(c p) s -> c p s", p=128)  # [6,128,4096]
    xT_v = xT_d.rearrange("(c p) t -> c p t", p=128)  # [4,128,256]
    wqT_v = wqT_d.rearrange("(c p) m -> c p m", p=128)  # [4,128,512]
    wkvT_v = wkvT_d.rearrange("(c p) m -> c p m", p=128)  # [6,128,1024]
    woT_v = woT_d.rearrange("(c p) m -> c p m", p=128)  # [4,128,512]

    with tile.TileContext(nc) as tc, ExitStack() as ctx:
        const = ctx.enter_context(tc.tile_pool(name="const", bufs=1))
        work = ctx.enter_context(tc.tile_pool(name="work", bufs=2))
        p_pe = ctx.enter_context(tc.tile_pool(name="p_pe", bufs=3, space="PSUM"))
        p_pv = ctx.enter_context(tc.tile_pool(name="p_pv", bufs=2, space="PSUM"))

        # ---- static SBUF tensors -------------------------------------------
        # ctx quarters: ctx_t[c][q] = [128, 1024]
        ctx_t = [
            [
                const.tile([128, 1024], BF16, tag=f"ctx{c}_{q}", name=f"ctx{c}_{q}")
                for q in range(4)
            ]
            for c in range(6)
        ]
        kt_t = [
            const.tile([128, S], BF16, tag=f"kt{kc}", name=f"kt{kc}") for kc in range(4)
        ]
        vp_t = [
            const.tile([128, 8 * 65], BF16, tag=f"vp{sc}", name=f"vp{sc}")
            for sc in range(32)
        ]
        qt_t = [
            const.tile([128, T], BF16, tag=f"qt{qc}", name=f"qt{qc}") for qc in range(4)
        ]
        ot_t = [
            const.tile([128, T], BF16, tag=f"ot{cc}", name=f"ot{cc}") for cc in range(4)
        ]
        wq_t = [
            const.tile([128, 512], BF16, tag=f"wq{ec}", name=f"wq{ec}")
            for ec in range(4)
        ]
        wkv_t = [
            const.tile([128, 1024], BF16, tag=f"wkv{c}", name=f"wkv{c}")
            for c in range(6)
        ]
        wo_t = [
            const.tile([128, 512], BF16, tag=f"wo{cc}", name=f"wo{cc}")
            for cc in range(4)
        ]
        x_t = [
            const.tile([128, T], BF16, tag=f"x{ec}", name=f"x{ec}") for ec in range(4)
        ]
        pvacc_t = [
            const.tile([65, T], F32, tag=f"pvacc{h}", name=f"pvacc{h}") for h in range(8)
        ]
        den8_t = const.tile([8, T], F32, tag="den8")
        rec8_t = const.tile([8, T], F32, tag="rec8")
        rech_t = const.tile([1, 8 * T], F32, tag="rech")
        m01_t = const.tile([128, 32], F32, tag="m01")
        bo_t = const.tile([128, 4], F32, tag="bo")
        ones8_t = const.tile([128, 8], BF16, tag="ones8")
        ones64_t = const.tile([1, 64], F32, tag="ones64")

        # ---- loads ----------------------------------------------------------
        nc.vector.memset(ones8_t[:], 1.0)
        nc.vector.memset(ones64_t[:], 1.0)
        for ec in range(4):
            nc.gpsimd.dma_start(x_t[ec][:], xT_v[ec])
            nc.gpsimd.dma_start(wq_t[ec][:], wqT_v[ec])
        for c in range(6):
            nc.gpsimd.dma_start(wkv_t[c][:], wkvT_v[c])
        for c in range(3):  # first ctx quarter split across both DMA queues
            nc.sync.dma_start(ctx_t[c][0][:], ctxT_v[c][:, 0:1024])
        for c in range(3, 6):
            nc.gpsimd.dma_start(ctx_t[c][0][:], ctxT_v[c][:, 0:1024])
        for q in range(1, 4):
            for c in range(6):
                nc.sync.dma_start(
                    ctx_t[c][q][:], ctxT_v[c][:, q * 1024 : (q + 1) * 1024]
                )
        nc.gpsimd.dma_start(m01_t[:], m01_d)
        for cc in range(4):
            nc.gpsimd.dma_start(wo_t[cc][:], woT_v[cc])
        nc.gpsimd.dma_start(bo_t[:], bo_d)

        def ctx_slice(c, s0, n):
            q = s0 // 1024
            off = s0 - q * 1024
            return ctx_t[c][q][:, off : off + n]

        # ---- Q projection ---------------------------------------------------
        for qc in range(4):
            ps = p_pe.tile([128, 1024], F32, tag="pe")
            for ec in range(4):
                nc.tensor.matmul(
                    ps[:, 0:T],
                    lhsT=wq_t[ec][:, qc * 128 : (qc + 1) * 128],
                    rhs=x_t[ec][:],
                    start=(ec == 0),
                    stop=(ec == 3),
                )
            nc.vector.tensor_copy(qt_t[qc][:], ps[:, 0:T])

        # ---- interleaved KV projection + attention, per ctx quarter ---------
        # Attention group (kc, g) only needs ctx quarter g//2, so scores/exp/PV
        # for s-chunks of quarter q run right after that quarter's K/V proj.
        # PV accumulates per-quarter in PSUM, then adds into SBUF pvacc (DVE),
        # keeping only 2 PV psum banks live and the ACT exp work overlapped
        # with the next quarter's kv-proj matmuls.
        for q in range(4):
            for kc in range(4):
                ps = p_pe.tile([128, 1024], F32, tag="pe")
                for c in range(6):
                    for halfi in range(2):
                        nc.tensor.matmul(
                            ps[:, halfi * 512 : (halfi + 1) * 512],
                            lhsT=wkv_t[c][:, kc * 128 : (kc + 1) * 128],
                            rhs=ctx_slice(c, q * 1024 + halfi * 512, 512),
                            start=(c == 0),
                            stop=(c == 5),
                        )
                nc.vector.tensor_copy(
                    kt_t[kc][:, q * 1024 : (q + 1) * 1024], ps[:]
                )
            for sc in range(q * 8, (q + 1) * 8):
                ps = p_pe.tile([128, 1024], F32, tag="pe")
                for c in range(6):
                    nc.tensor.matmul(
                        ps[:, 0:512],
                        lhsT=ctx_slice(c, sc * 128, 128),
                        rhs=wkv_t[c][:, 512:1024],
                        start=(c == 0),
                        stop=(c == 5),
                    )
                dst = vp_t[sc][:].rearrange("p (h e) -> p h e", e=65)
                nc.vector.tensor_scalar_mul(
                    dst[:, :, 0:64],
                    ps[:, 0:512].rearrange("p (h d) -> p h d", d=64),
                    m01_t[:, sc : sc + 1],
                )
                nc.vector.tensor_scalar_mul(
                    dst[:, :, 64:65],
                    ones8_t[:].rearrange("p (h o) -> p h o", o=1),
                    m01_t[:, sc : sc + 1],
                )
            for kc in range(4):
                pvq0 = p_pv.tile([65, T], F32, tag="pv")
                pvq1 = p_pv.tile([65, T], F32, tag="pv")
                for g in (2 * q, 2 * q + 1):
                    pe0 = p_pe.tile([128, 1024], F32, tag="pe")
                    pe1 = p_pe.tile([128, 1024], F32, tag="pe")
                    for j in range(4):
                        sc = g * 4 + j
                        nc.tensor.matmul(
                            pe0[:, j * 256 : (j + 1) * 256],
                            lhsT=kt_t[kc][0:64, sc * 128 : (sc + 1) * 128],
                            rhs=qt_t[kc][0:64, :],
                            start=True,
                            stop=True,
                        )
                        nc.tensor.matmul(
                            pe1[:, j * 256 : (j + 1) * 256],
                            lhsT=kt_t[kc][64:128, sc * 128 : (sc + 1) * 128],
                            rhs=qt_t[kc][64:128, :],
                            start=True,
                            stop=True,
                        )
                    e0 = work.tile([128, 1024], BF16, tag="exp", bufs=6)
                    nc.scalar.activation(
                        e0[:], pe0[:], mybir.ActivationFunctionType.Exp
                    )
                    e1 = work.tile([128, 1024], BF16, tag="exp", bufs=6)
                    nc.scalar.activation(
                        e1[:], pe1[:], mybir.ActivationFunctionType.Exp
                    )
                    for j in range(4):
                        sc = g * 4 + j
                        nc.tensor.matmul(
                            pvq0[:],
                            lhsT=vp_t[sc][:, (2 * kc) * 65 : (2 * kc) * 65 + 65],
                            rhs=e0[:, j * 256 : (j + 1) * 256],
                            start=(g == 2 * q and j == 0),
                            stop=(g == 2 * q + 1 and j == 3),
                        )
                        nc.tensor.matmul(
                            pvq1[:],
                            lhsT=vp_t[sc][
                                :, (2 * kc + 1) * 65 : (2 * kc + 1) * 65 + 65
                            ],
                            rhs=e1[:, j * 256 : (j + 1) * 256],
                            start=(g == 2 * q and j == 0),
                            stop=(g == 2 * q + 1 and j == 3),
                        )
                if q == 0:
                    nc.vector.tensor_copy(pvacc_t[2 * kc][:], pvq0[:])
                    nc.vector.tensor_copy(pvacc_t[2 * kc + 1][:], pvq1[:])
                else:
                    nc.vector.tensor_add(
                        pvacc_t[2 * kc][:], pvacc_t[2 * kc][:], pvq0[:]
                    )
                    nc.vector.tensor_add(
                        pvacc_t[2 * kc + 1][:], pvacc_t[2 * kc + 1][:], pvq1[:]
                    )
                if q == 3:
                    nc.sync.dma_start(
                        den8_t[2 * kc : 2 * kc + 1, :], pvacc_t[2 * kc][64:65, :]
                    )
                    nc.gpsimd.dma_start(
                        den8_t[2 * kc + 1 : 2 * kc + 2, :],
                        pvacc_t[2 * kc + 1][64:65, :],
                    )

        # ---- deferred softmax normalization (off the PE critical path) ------
        nc.vector.reciprocal(rec8_t[:], den8_t[:])
        nc.sync.dma_start(
            rech_t[0:1, :].rearrange("p (h t) -> p h t", t=T), rec8_t[:, :]
        )
        for kc in range(4):
            bc0 = p_pv.tile([64, T], F32, tag="pv")
            nc.tensor.matmul(
                bc0[:],
                lhsT=ones64_t[:],
                rhs=rech_t[0:1, (2 * kc) * T : (2 * kc + 1) * T],
                start=True,
                stop=True,
            )
            bc1 = p_pv.tile([64, T], F32, tag="pv")
            nc.tensor.matmul(
                bc1[:],
                lhsT=ones64_t[:],
                rhs=rech_t[0:1, (2 * kc + 1) * T : (2 * kc + 2) * T],
                start=True,
                stop=True,
            )
            nc.vector.tensor_mul(ot_t[kc][0:64, :], pvacc_t[2 * kc][0:64, :], bc0[:])
            tmp1 = work.tile([64, T], BF16, tag="otmp", bufs=2)
            nc.vector.tensor_mul(tmp1[:], pvacc_t[2 * kc + 1][0:64, :], bc1[:])
            nc.sync.dma_start(ot_t[kc][64:128, :], tmp1[:])

        # ---- out projection -------------------------------------------------
        for eo in range(4):
            ps = p_pe.tile([128, 1024], F32, tag="pe")
            for cc in range(4):
                nc.tensor.matmul(
                    ps[:, 0:T],
                    lhsT=wo_t[cc][:, eo * 128 : (eo + 1) * 128],
                    rhs=ot_t[cc][:],
                    start=(cc == 0),
                    stop=(cc == 3),
                )
            osb = work.tile([128, T], F32, tag="osb", bufs=2)
            nc.vector.tensor_scalar_add(osb[:], ps[:, 0:T], bo_t[:, eo : eo + 1])
            nc.sync.dma_start(outT_d[eo], osb[:])

    nc.compile()
    return nc


_NC = None


def _get_nc():
    global _NC
    if _NC is None:
        _NC = _build_program()
    return _NC


def _prep_in_maps(x, context, key_padding_mask, Wq, Wkv, Wo, bo):
    wqT = (np.ascontiguousarray(Wq.T) * np.float32(D**-0.5)).astype(NPBF16)
    wkvT = np.ascontiguousarray(Wkv.T).astype(NPBF16)
    woT = np.ascontiguousarray(Wo.T).astype(NPBF16)
    bo_r = np.ascontiguousarray(bo.reshape(4, 128).T).astype(np.float32)
    in_maps = []
    for b in range(B):
        ctxT = np.ascontiguousarray(context[b].T).astype(NPBF16)
        xT = np.ascontiguousarray(x[b].T).astype(NPBF16)
        m01 = np.ascontiguousarray(
            (~key_padding_mask[b]).astype(np.float32).reshape(32, 128).T
        )
        in_maps.append(
            dict(ctxT=ctxT, xT=xT, m01=m01, wqT=wqT, wkvT=wkvT, woT=woT, bo_r=bo_r)
        )
    return in_maps


def _run(inputs, trace=False, **kw):
    nc = _get_nc()
    in_maps = _prep_in_maps(**inputs)
    res = bass_utils.run_bass_kernel_spmd(
        nc, in_maps, core_ids=list(range(NC_CORES)), trace=trace, **kw
    )
    out = np.stack(
        [res.results[b]["outT"].reshape(E, T).T for b in range(B)]
    ).astype(np.float32)
    return out, res


def kernel(**inputs):
    out, _ = _run(inputs, trace=False)
    return out


if __name__ == "__main__":
    rng = np.random.default_rng(0)
    ins = dict(
        x=rng.standard_normal((B, T, E), dtype=np.float32),
        context=rng.standard_normal((B, S, KV), dtype=np.float32),
        key_padding_mask=rng.integers(0, 2, (B, S)).astype(bool),
        Wq=(rng.standard_normal((512, E), dtype=np.float32) * 0.02),
        Wkv=(rng.standard_normal((1024, KV), dtype=np.float32) * 0.02),
        Wo=(rng.standard_normal((E, 512), dtype=np.float32) * 0.02),
        bo=np.zeros(E, dtype=np.float32),
    )
    out = kernel(**ins)
    print("out", out.shape, out.dtype, np.abs(out).mean())



# revision 26
# speedup vs baseline: 1.5345x; 1.5345x over previous
"""CrossAttention Trainium2 kernel, v2.

Problem (hardcoded): B=8, T=256, S=4096, E=512, KV=768, H=8, D=64.
Sharding: data-parallel over B — one batch per NeuronCore (8 cores).

Key ideas vs v1:
  * Key compaction: ~50% of keys are masked. Host gathers kept keys per
    batch and pads to S_C = ceil(max_kept/128)*128 (2176 for the bench
    seed). All S-proportional work (KV proj, scores, exp, PV) shrinks
    by ~47%. Pad rows have zero context columns -> k=0 -> score 0 ->
    exp 1, but vp rows and the ones-column are 0 so they contribute
    nothing to numerator or denominator (m01 carries kept/pad only).
  * PV accumulated directly in PSUM across ALL s-chunks (no per-quarter
    DVE adds): 8 heads packed as 4 banks of [65, 512] (head pair side
    by side along free dim; row 64 = softmax denominator).
  * Group-pipelined emission: ctx arrives in groups of 4 s-chunks;
    K/V projection of group g is interleaved instruction-by-instruction
    with attention of group g-1 so the PE never waits on the (scalar
    engine) exp, and exp/psum-cast engine work spreads evenly.
  * Per-head-pair normalization (reciprocal -> K=1 broadcast matmul ->
    DVE muls) starts as soon as that pair's last PV lands; the out
    projection contracts in 64-row chunks so each pair feeds it
    immediately. Tail is a few us instead of ~20.
"""

import os
import sys

sys.path.insert(0, "/opt/trn_rl_repo")

# Debug truncation: 1=proj, 2=+scores, 3=+exp, 4=+pv, 5=+norm, 6=full (default)
V2_STAGE = int(os.environ.get("V2_STAGE", "6"))
V2_NOILV = os.environ.get("V2_NOILV", "") == "1"  # emit proj before attn (no interleave)

import numpy as np
import ml_dtypes
from contextlib import ExitStack

import concourse.bass as bass
import concourse.bacc as bacc
import concourse.tile as tile
from concourse import mybir
from concourse import bass_utils

BF16 = mybir.dt.bfloat16
F32 = mybir.dt.float32
NPBF16 = ml_dtypes.bfloat16

B, T, S, E, KV, H, D = 8, 256, 4096, 512, 768, 8, 64
NC_CORES = 8
GROUP_SC = 4  # s-chunks per pipeline group


def _build_program(n_sc):
    S_C = n_sc * 128
    groups = []
    sc0 = 0
    while sc0 < n_sc:
        n = min(GROUP_SC, n_sc - sc0)
        groups.append((sc0, n))
        sc0 += n
    G = len(groups)

    nc = bacc.Bacc("TRN2", target_bir_lowering=False, debug=False)

    ctxb_d = nc.dram_tensor("ctxb", [128, 6 * S_C], BF16, kind="ExternalInput").ap()
    x_d = nc.dram_tensor("xr", [128, 4 * T], BF16, kind="ExternalInput").ap()
    m01_d = nc.dram_tensor("m01", [128, n_sc], F32, kind="ExternalInput").ap()
    wq_d = nc.dram_tensor("wqr", [128, 4 * 512], BF16, kind="ExternalInput").ap()
    wkvk_d = nc.dram_tensor("wkvk", [128, 6 * 512], BF16, kind="ExternalInput").ap()
    wkvv_d = nc.dram_tensor("wkvv", [128, 6 * 512], BF16, kind="ExternalInput").ap()
    wo_d = nc.dram_tensor("wo64", [64, 8 * 512], BF16, kind="ExternalInput").ap()
    bo_d = nc.dram_tensor("bo_r", [128, 4], F32, kind="ExternalInput").ap()
    outT_d = nc.dram_tensor("outT", [4, 128, T], F32, kind="ExternalOutput").ap()

    with tile.TileContext(nc) as tc, ExitStack() as ctx:
        const = ctx.enter_context(tc.tile_pool(name="const", bufs=1))
        work = ctx.enter_context(tc.tile_pool(name="work", bufs=2))
        psum = ctx.enter_context(tc.tile_pool(name="psum", bufs=1, space="PSUM"))

        # ---- static SBUF tensors ------------------------------------------
        ctxb_t = const.tile([128, 6 * S_C], BF16, tag="ctxb", name="ctxb")
        x_t = const.tile([128, 4 * T], BF16, tag="x", name="x")
        wq_t = const.tile([128, 4 * 512], BF16, tag="wq", name="wq")
        wkvk_t = const.tile([128, 6 * 512], BF16, tag="wkvk", name="wkvk")
        wkvv_t = const.tile([128, 6 * 512], BF16, tag="wkvv", name="wkvv")
        wo_t = const.tile([64, 8 * 512], BF16, tag="wo", name="wo")
        bo_t = const.tile([128, 4], F32, tag="bo", name="bo")
        m01_t = const.tile([128, n_sc], F32, tag="m01", name="m01")
        kt_t = [
            const.tile([128, S_C], BF16, tag=f"kt{kc}", name=f"kt{kc}")
            for kc in range(4)
        ]
        vp_t = [
            const.tile([128, 8 * 65], BF16, tag=f"vp{sc}", name=f"vp{sc}")
            for sc in range(n_sc)
        ]
        qt_t = [
            const.tile([128, T], BF16, tag=f"qt{qc}", name=f"qt{qc}") for qc in range(4)
        ]
        otE_t = [
            const.tile([64, T], BF16, tag=f"otE{kc}", name=f"otE{kc}")
            for kc in range(4)
        ]
        otO_t = [
            const.tile([64, T], BF16, tag=f"otO{kc}", name=f"otO{kc}")
            for kc in range(4)
        ]
        ones8_t = const.tile([128, 8], BF16, tag="ones8", name="ones8")
        ones64_t = const.tile([1, 64], BF16, tag="ones64", name="ones64")
        denp_t = const.tile([1, 4 * 512], F32, tag="denp", name="denp")
        rechp_t = const.tile([1, 4 * 512], BF16, tag="rechp", name="rechp")
        dummy_t = const.tile([1, 64], BF16, tag="dummy", name="dummy")

        # SBUF accumulators for PV (head pair packed along free; row 64 = denom)
        pvacc_t = [
            const.tile([65, 512], F32, tag=f"pvacc{kc}", name=f"pvacc{kc}")
            for kc in range(4)
        ]

        # ---- tiny init + ACT table warm -----------------------------------
        nc.vector.memset(ones8_t[:], 1.0)
        nc.vector.memset(ones64_t[:], 1.0)
        nc.scalar.activation(
            dummy_t[:], ones64_t[:], mybir.ActivationFunctionType.Exp
        )

        # ---- DMA dispatches (two hw queues; priority order) ---------------
        nc.sync.dma_start(x_t[:], x_d)
        nc.sync.dma_start(wq_t[:], wq_d)
        nc.sync.dma_start(wkvk_t[:], wkvk_d)
        nc.gpsimd.dma_start(wkvv_t[:], wkvv_d)
        nc.gpsimd.dma_start(m01_t[:], m01_d)
        ctxb_v = ctxb_t[:].rearrange("p (c s) -> p c s", c=6)
        ctxd_v = ctxb_d.rearrange("p (c s) -> p c s", c=6)
        for gi, (g0, gn) in enumerate(groups):
            cols = slice(g0 * 128, g0 * 128 + gn * 128)
            q = nc.sync if gi % 2 == 0 else nc.gpsimd
            for c in range(6):
                q.dma_start(ctxb_v[:, c, cols], ctxd_v[:, c, cols])
        nc.gpsimd.dma_start(wo_t[:], wo_d)
        nc.gpsimd.dma_start(bo_t[:], bo_d)

        # ---- Q projection (first PE work, needs only x+wq) ----------------
        for qc in range(4):
            ps = psum.tile([128, T], F32, tag="w", bufs=2, name="qps")
            for ec in range(4):
                nc.tensor.matmul(
                    ps[:],
                    lhsT=wq_t[:, ec * 512 + qc * 128 : ec * 512 + (qc + 1) * 128],
                    rhs=x_t[:, ec * T : (ec + 1) * T],
                    start=(ec == 0),
                    stop=(ec == 3),
                )
            nc.scalar.copy(qt_t[qc][:], ps[:])

        # ---- pipeline: proj(g) interleaved with attn(g-1) -----------------
        def proj_ops(gi):
            g0, gn = groups[gi]
            cols = slice(g0 * 128, g0 * 128 + gn * 128)
            ops = []
            for kc in range(4):

                def k_proj(kc=kc, cols=cols, gn=gn, g0=g0):
                    ps = psum.tile([128, 512], F32, tag="w", bufs=2, name="kps")
                    for c in range(6):
                        nc.tensor.matmul(
                            ps[:, 0 : gn * 128],
                            lhsT=wkvk_t[
                                :, c * 512 + kc * 128 : c * 512 + (kc + 1) * 128
                            ],
                            rhs=ctxb_v[:, c, cols],
                            start=(c == 0),
                            stop=(c == 5),
                        )
                    nc.scalar.copy(kt_t[kc][:, cols], ps[:, 0 : gn * 128])

                ops.append(k_proj)
            for sc in range(g0, g0 + gn):

                def v_proj(sc=sc):
                    ps = psum.tile([128, 512], F32, tag="w", bufs=2, name="vps")
                    for c in range(6):
                        nc.tensor.matmul(
                            ps[:],
                            lhsT=ctxb_v[:, c, sc * 128 : (sc + 1) * 128],
                            rhs=wkvv_t[:, c * 512 : (c + 1) * 512],
                            start=(c == 0),
                            stop=(c == 5),
                        )
                    dst = vp_t[sc][:].rearrange("p (h e) -> p h e", e=65)
                    nc.vector.tensor_copy(
                        dst[:, :, 0:64], ps[:].rearrange("p (h d) -> p h d", d=64)
                    )
                    nc.vector.tensor_scalar_mul(
                        dst[:, :, 64:65],
                        ones8_t[:].rearrange("p (h o) -> p h o", o=1),
                        m01_t[:, sc : sc + 1],
                    )

                ops.append(v_proj)
            return ops

        def attn_units(gi):
            g0, gn = groups[gi]
            pairs = []
            p0 = g0
            while p0 < g0 + gn:
                np_ = min(2, g0 + gn - p0)
                pairs.append((p0, np_))
                p0 += np_
            return [(kc, p0, np_) for kc in range(4) for (p0, np_) in pairs]

        pend_pv = []  # [(kc, p0, np_, eE, eO)]
        n_done = [0] * 4  # PV s-chunks accumulated per kc
        pvq = [None] * 4  # current per-group psum pair for each kc

        def sc_group(sc):
            for g0, gn in groups:
                if g0 <= sc < g0 + gn:
                    return g0, gn
            raise AssertionError

        def emit_scores(kc, p0, np_):
            if V2_STAGE < 2:
                return
            # separate psum tiles per head: row-group-0 and row-group-64
            # outputs must not share a psum bank
            psE = psum.tile([128, 512], F32, tag="sc", bufs=2, name="psE")
            psO = psum.tile([128, 512], F32, tag="sc", bufs=2, name="psO")
            for i in range(np_):
                sc = p0 + i
                nc.tensor.matmul(
                    psE[:, i * T : (i + 1) * T],
                    lhsT=kt_t[kc][0:64, sc * 128 : (sc + 1) * 128],
                    rhs=qt_t[kc][0:64, :],
                    start=True,
                    stop=True,
                )
            for i in range(np_):
                sc = p0 + i
                nc.tensor.matmul(
                    psO[:, i * T : (i + 1) * T],
                    lhsT=kt_t[kc][64:128, sc * 128 : (sc + 1) * 128],
                    rhs=qt_t[kc][64:128, :],
                    start=True,
                    stop=True,
                )
            if V2_STAGE < 3:
                return
            eE = work.tile([128, 512], BF16, tag="e", bufs=6, name="eE")
            eO = work.tile([128, 512], BF16, tag="e", bufs=6, name="eO")
            nc.scalar.activation(
                eE[:, 0 : np_ * T], psE[:, 0 : np_ * T],
                mybir.ActivationFunctionType.Exp,
            )
            nc.scalar.activation(
                eO[:, 0 : np_ * T], psO[:, 0 : np_ * T],
                mybir.ActivationFunctionType.Exp,
            )
            if V2_STAGE < 4:
                return
            pend_pv.append((kc, p0, np_, eE, eO))

        def emit_pv():
            kc, p0, np_, eE, eO = pend_pv.pop(0)
            g0, gn = sc_group(p0)
            if p0 == g0:
                pvq[kc] = [
                    psum.tile([65, T], F32, tag="pv", bufs=4, name=f"pvq{hi}")
                    for hi in range(2)
                ]
            for i in range(np_):
                sc = p0 + i
                for hi, e in ((0, eE), (1, eO)):
                    h = 2 * kc + hi
                    nc.tensor.matmul(
                        pvq[kc][hi][:],
                        lhsT=vp_t[sc][:, h * 65 : h * 65 + 65],
                        rhs=e[:, i * T : (i + 1) * T],
                        start=(sc == g0),
                        stop=(sc == g0 + gn - 1),
                    )
            if p0 + np_ == g0 + gn:
                # fold the group's PV into the SBUF accumulator (DVE)
                for hi in range(2):
                    dst = pvacc_t[kc][:, hi * T : (hi + 1) * T]
                    if g0 == 0:
                        nc.vector.tensor_copy(dst, pvq[kc][hi][:])
                    else:
                        nc.vector.tensor_add(dst, dst, pvq[kc][hi][:])
            n_done[kc] += np_
            if n_done[kc] == n_sc and V2_STAGE >= 5:
                emit_norm(kc)

        def emit_norm(kc):
            # denominators live in pvacc row 64 (cols 0:256 headE, 256:512 headO)
            nc.sync.dma_start(
                denp_t[0:1, kc * 512 : (kc + 1) * 512], pvacc_t[kc][64:65, :]
            )
            with nc.allow_low_precision(reason="softmax denom reciprocal in bf16"):
                nc.vector.reciprocal(
                    rechp_t[0:1, kc * 512 : (kc + 1) * 512],
                    denp_t[0:1, kc * 512 : (kc + 1) * 512],
                )
            bc = psum.tile([128, 512], F32, tag="sc", bufs=2, name="bc")
            nc.tensor.matmul(
                bc[0:64, :],
                lhsT=ones64_t[:],
                rhs=rechp_t[0:1, kc * 512 : (kc + 1) * 512],
                start=True,
                stop=True,
            )
            nc.vector.tensor_mul(otE_t[kc][:], pvacc_t[kc][0:64, 0:T], bc[0:64, 0:T])
            nc.vector.tensor_mul(
                otO_t[kc][:], pvacc_t[kc][0:64, T : 2 * T], bc[0:64, T : 2 * T]
            )
            if V2_STAGE < 6:
                return
            # out projection contribution of this head pair (64-row chunks)
            for j, ot in ((2 * kc, otE_t[kc]), (2 * kc + 1, otO_t[kc])):
                for oi in range(4):
                    nc.tensor.matmul(
                        out_ps[oi // 2][:, (oi % 2) * T : (oi % 2 + 1) * T],
                        lhsT=wo_t[:, j * 512 + oi * 128 : j * 512 + (oi + 1) * 128],
                        rhs=ot[:],
                        start=(j == 0 and oi % 2 == 0),
                        stop=(j == 7 and oi % 2 == 1),
                    )

        out_ps = None
        for gi in range(G + 1):
            units = attn_units(gi - 1) if gi >= 1 else []
            pops = proj_ops(gi) if gi < G else []
            if gi == G:
                # allocate out-projection accumulators (w-ring is free now)
                out_ps = [
                    psum.tile([128, 512], F32, tag="w", bufs=2, name=f"ops{i}")
                    for i in range(2)
                ]
            if V2_STAGE < 2:
                units = []
            if not units:
                for op in pops:
                    op()
                continue
            # interleave proj ops between attention units
            if V2_NOILV:
                for op in pops:
                    op()
                pops = []
            k = 0
            for i, (kc, p0, np_) in enumerate(units):
                emit_scores(kc, p0, np_)
                while len(pend_pv) > 1:
                    emit_pv()
                k_to = (i + 1) * len(pops) // len(units) if pops else 0
                while k < k_to:
                    pops[k]()
                    k += 1
        while pend_pv:
            emit_pv()

        # ---- bias + store -------------------------------------------------
        for half in range(2):
            osb = work.tile([128, 512], F32, tag="osb", bufs=2, name="osb")
            if V2_STAGE >= 6:
                for eo2 in range(2):
                    eo = half * 2 + eo2
                    nc.vector.tensor_scalar_add(
                        osb[:, eo2 * T : (eo2 + 1) * T],
                        out_ps[half][:, eo2 * T : (eo2 + 1) * T],
                        bo_t[:, eo : eo + 1],
                    )
            else:
                nc.vector.memset(osb[:], 0.0)
            for eo2 in range(2):
                q = nc.sync if eo2 == 0 else nc.gpsimd
                q.dma_start(
                    outT_d[2 * half + eo2], osb[:, eo2 * T : (eo2 + 1) * T]
                )

    nc.compile()
    return nc


_NC = {}


def _get_nc(n_sc):
    if n_sc not in _NC:
        _NC[n_sc] = _build_program(n_sc)
    return _NC[n_sc]


def _prep_in_maps(x, context, key_padding_mask, Wq, Wkv, Wo, bo):
    keep = ~np.asarray(key_padding_mask)
    kept = keep.sum(axis=1)
    n_sc = max(1, -(-int(kept.max()) // 128))
    S_C = n_sc * 128

    scale = np.float32(D**-0.5)
    wqr = (
        (np.ascontiguousarray(Wq.T) * scale)
        .reshape(4, 128, 4 * 128)
        .transpose(1, 0, 2)
        .reshape(128, 4 * 512)
        .astype(NPBF16)
    )
    wkvT = np.ascontiguousarray(Wkv.T)  # [768, 1024]
    wkvk = (
        wkvT[:, :512].reshape(6, 128, 512).transpose(1, 0, 2).reshape(128, 6 * 512)
    ).astype(NPBF16)
    wkvv = (
        wkvT[:, 512:].reshape(6, 128, 512).transpose(1, 0, 2).reshape(128, 6 * 512)
    ).astype(NPBF16)
    wo64 = (
        np.ascontiguousarray(Wo.T)
        .reshape(8, 64, 512)
        .transpose(1, 0, 2)
        .reshape(64, 8 * 512)
    ).astype(NPBF16)
    bo_r = np.ascontiguousarray(bo.reshape(4, 128).T).astype(np.float32)

    in_maps = []
    for b in range(B):
        idx = np.nonzero(keep[b])[0]
        k_b = len(idx)
        ctxT = np.zeros((KV, S_C), np.float32)
        ctxT[:, :k_b] = context[b][idx].T
        ctxb = (
            ctxT.reshape(6, 128, S_C).transpose(1, 0, 2).reshape(128, 6 * S_C)
        ).astype(NPBF16)
        xr = (
            np.ascontiguousarray(x[b].T)
            .reshape(4, 128, T)
            .transpose(1, 0, 2)
            .reshape(128, 4 * T)
        ).astype(NPBF16)
        m01 = np.zeros((128, n_sc), np.float32)
        flat = np.arange(S_C) < k_b
        m01[:, :] = flat.reshape(n_sc, 128).T
        in_maps.append(
            dict(
                ctxb=ctxb,
                xr=xr,
                m01=m01,
                wqr=wqr,
                wkvk=wkvk,
                wkvv=wkvv,
                wo64=wo64,
                bo_r=bo_r,
            )
        )
    return in_maps, n_sc


def _run(inputs, trace=False, **kw):
    in_maps, n_sc = _prep_in_maps(**inputs)
    nc = _get_nc(n_sc)
    res = bass_utils.run_bass_kernel_spmd(
        nc, in_maps, core_ids=list(range(NC_CORES)), trace=trace, **kw
    )
    out = np.stack(
        [res.results[b]["outT"].reshape(E, T).T for b in range(B)]
    ).astype(np.float32)
    return out, res


def kernel(**inputs):
    out, _ = _run(inputs, trace=False)
    return out


if __name__ == "__main__":
    rng = np.random.default_rng(0)
    ins = dict(
        x=rng.standard_normal((B, T, E), dtype=np.float32),
        context=rng.standard_normal((B, S, KV), dtype=np.float32),
        key_padding_mask=rng.integers(0, 2, (B, S)).astype(bool),
        Wq=(rng.standard_normal((512, E), dtype=np.float32) * 0.02),
        Wkv=(rng.standard_normal((1024, KV), dtype=np.float32) * 0.02),
        Wo=(rng.standard_normal((E, 512), dtype=np.float32) * 0.02),
        bo=np.zeros(E, dtype=np.float32),
    )
    out = kernel(**ins)
    print("out", out.shape, out.dtype, np.abs(out).mean())


# revision 27
# speedup vs baseline: 1.5625x; 1.0182x over previous
"""CrossAttention Trainium2 kernel, v2.

Problem (hardcoded): B=8, T=256, S=4096, E=512, KV=768, H=8, D=64.
Sharding: data-parallel over B — one batch per NeuronCore (8 cores).

Key ideas vs v1:
  * Key compaction: ~50% of keys are masked. Host gathers kept keys per
    batch and pads to S_C = ceil(max_kept/128)*128 (2176 for the bench
    seed). All S-proportional work (KV proj, scores, exp, PV) shrinks
    by ~47%. Pad rows have zero context columns -> k=0 -> score 0 ->
    exp 1, but vp rows and the ones-column are 0 so they contribute
    nothing to numerator or denominator (m01 carries kept/pad only).
  * PV accumulated directly in PSUM across ALL s-chunks (no per-quarter
    DVE adds): 8 heads packed as 4 banks of [65, 512] (head pair side
    by side along free dim; row 64 = softmax denominator).
  * Group-pipelined emission: ctx arrives in groups of 4 s-chunks;
    K/V projection of group g is interleaved instruction-by-instruction
    with attention of group g-1 so the PE never waits on the (scalar
    engine) exp, and exp/psum-cast engine work spreads evenly.
  * Per-head-pair normalization (reciprocal -> K=1 broadcast matmul ->
    DVE muls) starts as soon as that pair's last PV lands; the out
    projection contracts in 64-row chunks so each pair feeds it
    immediately. Tail is a few us instead of ~20.
"""

import os
import sys

sys.path.insert(0, "/opt/trn_rl_repo")

# Debug truncation: 1=proj, 2=+scores, 3=+exp, 4=+pv, 5=+norm, 6=full (default)
V2_STAGE = int(os.environ.get("V2_STAGE", "6"))
V2_NOILV = os.environ.get("V2_NOILV", "") == "1"  # emit proj before attn (no interleave)

import numpy as np
import ml_dtypes
from contextlib import ExitStack

import concourse.bass as bass
import concourse.bacc as bacc
import concourse.tile as tile
from concourse import mybir
from concourse import bass_utils

BF16 = mybir.dt.bfloat16
F32 = mybir.dt.float32
NPBF16 = ml_dtypes.bfloat16

B, T, S, E, KV, H, D = 8, 256, 4096, 512, 768, 8, 64
NC_CORES = 8
GROUP_SC = 4  # s-chunks per pipeline group


def _build_program(n_sc):
    S_C = n_sc * 128
    sizes = []
    rem = n_sc
    while rem > 5:
        sizes.append(GROUP_SC)
        rem -= GROUP_SC
    while rem > 0:
        n = 2 if rem > 2 else rem
        sizes.append(n)
        rem -= n
    groups = []
    sc0 = 0
    for n in sizes:
        groups.append((sc0, n))
        sc0 += n
    G = len(groups)

    nc = bacc.Bacc("TRN2", target_bir_lowering=False, debug=False)

    ctxb_d = nc.dram_tensor("ctxb", [128, 6 * S_C], BF16, kind="ExternalInput").ap()
    x_d = nc.dram_tensor("xr", [128, 4 * T], BF16, kind="ExternalInput").ap()
    m01_d = nc.dram_tensor("m01", [128, n_sc], F32, kind="ExternalInput").ap()
    wq_d = nc.dram_tensor("wqr", [128, 4 * 512], BF16, kind="ExternalInput").ap()
    wkvk_d = nc.dram_tensor("wkvk", [128, 6 * 512], BF16, kind="ExternalInput").ap()
    wkvv_d = nc.dram_tensor("wkvv", [128, 6 * 512], BF16, kind="ExternalInput").ap()
    wo_d = nc.dram_tensor("wo64", [64, 8 * 512], BF16, kind="ExternalInput").ap()
    bo_d = nc.dram_tensor("bo_r", [128, 4], F32, kind="ExternalInput").ap()
    outT_d = nc.dram_tensor("outT", [4, 128, T], F32, kind="ExternalOutput").ap()

    with tile.TileContext(nc) as tc, ExitStack() as ctx:
        const = ctx.enter_context(tc.tile_pool(name="const", bufs=1))
        work = ctx.enter_context(tc.tile_pool(name="work", bufs=2))
        psum = ctx.enter_context(tc.tile_pool(name="psum", bufs=1, space="PSUM"))

        # ---- static SBUF tensors ------------------------------------------
        ctxb_t = const.tile([128, 6 * S_C], BF16, tag="ctxb", name="ctxb")
        x_t = const.tile([128, 4 * T], BF16, tag="x", name="x")
        wq_t = const.tile([128, 4 * 512], BF16, tag="wq", name="wq")
        wkvk_t = const.tile([128, 6 * 512], BF16, tag="wkvk", name="wkvk")
        wkvv_t = const.tile([128, 6 * 512], BF16, tag="wkvv", name="wkvv")
        wo_t = const.tile([64, 8 * 512], BF16, tag="wo", name="wo")
        bo_t = const.tile([128, 4], F32, tag="bo", name="bo")
        m01_t = const.tile([128, n_sc], F32, tag="m01", name="m01")
        kt_t = [
            const.tile([128, S_C], BF16, tag=f"kt{kc}", name=f"kt{kc}")
            for kc in range(4)
        ]
        vp_t = [
            const.tile([128, 8 * 65], BF16, tag=f"vp{sc}", name=f"vp{sc}")
            for sc in range(n_sc)
        ]
        qt_t = [
            const.tile([128, T], BF16, tag=f"qt{qc}", name=f"qt{qc}") for qc in range(4)
        ]
        otE_t = [
            const.tile([64, T], BF16, tag=f"otE{kc}", name=f"otE{kc}")
            for kc in range(4)
        ]
        otO_t = [
            const.tile([64, T], BF16, tag=f"otO{kc}", name=f"otO{kc}")
            for kc in range(4)
        ]
        ones8_t = const.tile([128, 8], BF16, tag="ones8", name="ones8")
        ones64_t = const.tile([1, 64], BF16, tag="ones64", name="ones64")
        denp_t = const.tile([128, 16], F32, tag="denp", name="denp")
        recb_t = const.tile([128, 16], BF16, tag="recb", name="recb")
        rechp_t = const.tile([1, 4 * 512], BF16, tag="rechp", name="rechp")
        dummy_t = const.tile([1, 64], BF16, tag="dummy", name="dummy")

        # SBUF accumulators for PV (head pair packed along free; row 64 = denom)
        pvacc_t = [
            const.tile([65, 512], F32, tag=f"pvacc{kc}", name=f"pvacc{kc}")
            for kc in range(4)
        ]

        # ---- tiny init + ACT table warm -----------------------------------
        nc.vector.memset(ones8_t[:], 1.0)
        nc.vector.memset(ones64_t[:], 1.0)
        nc.scalar.activation(
            dummy_t[:], ones64_t[:], mybir.ActivationFunctionType.Exp
        )

        # ---- DMA dispatches (two hw queues; priority order) ---------------
        nc.sync.dma_start(x_t[:], x_d)
        nc.gpsimd.dma_start(wq_t[:], wq_d)
        nc.sync.dma_start(wkvk_t[:], wkvk_d)
        nc.gpsimd.dma_start(wkvv_t[:], wkvv_d)
        ctxb_v = ctxb_t[:].rearrange("p (c s) -> p c s", c=6)
        ctxd_v = ctxb_d.rearrange("p (c s) -> p c s", c=6)
        g0_, gn_ = groups[0]
        cols0 = slice(g0_ * 128, g0_ * 128 + gn_ * 128)
        for c in range(6):
            q = nc.sync if c < 3 else nc.gpsimd
            q.dma_start(ctxb_v[:, c, cols0], ctxd_v[:, c, cols0])
        nc.gpsimd.dma_start(m01_t[:], m01_d)
        for gi, (g0, gn) in enumerate(groups[1:], 1):
            cols = slice(g0 * 128, g0 * 128 + gn * 128)
            q = nc.sync if gi % 2 == 0 else nc.gpsimd
            for c in range(6):
                q.dma_start(ctxb_v[:, c, cols], ctxd_v[:, c, cols])
        nc.gpsimd.dma_start(wo_t[:], wo_d)
        nc.gpsimd.dma_start(bo_t[:], bo_d)

        # ---- Q projection (first PE work, needs only x+wq) ----------------
        for qc in range(4):
            ps = psum.tile([128, T], F32, tag="w", bufs=2, name="qps")
            for ec in range(4):
                nc.tensor.matmul(
                    ps[:],
                    lhsT=wq_t[:, ec * 512 + qc * 128 : ec * 512 + (qc + 1) * 128],
                    rhs=x_t[:, ec * T : (ec + 1) * T],
                    start=(ec == 0),
                    stop=(ec == 3),
                )
            nc.scalar.copy(qt_t[qc][:], ps[:])

        # ---- pipeline: proj(g) interleaved with attn(g-1) -----------------
        def proj_ops(gi):
            g0, gn = groups[gi]
            cols = slice(g0 * 128, g0 * 128 + gn * 128)
            ops = []
            for kc in range(4):

                def k_proj(kc=kc, cols=cols, gn=gn, g0=g0, gi=gi):
                    ps = psum.tile([128, 512], F32, tag="w", bufs=2, name="kps")
                    for c in range(6):
                        nc.tensor.matmul(
                            ps[:, 0 : gn * 128],
                            lhsT=wkvk_t[
                                :, c * 512 + kc * 128 : c * 512 + (kc + 1) * 128
                            ],
                            rhs=ctxb_v[:, c, cols],
                            start=(c == 0),
                            stop=(c == 5),
                        )
                    if gi % 2 == 0:
                        nc.scalar.copy(kt_t[kc][:, cols], ps[:, 0 : gn * 128])
                    else:
                        nc.vector.tensor_copy(kt_t[kc][:, cols], ps[:, 0 : gn * 128])

                ops.append(k_proj)
            for sc in range(g0, g0 + gn):

                def v_proj(sc=sc):
                    ps = psum.tile([128, 512], F32, tag="w", bufs=2, name="vps")
                    for c in range(6):
                        nc.tensor.matmul(
                            ps[:],
                            lhsT=ctxb_v[:, c, sc * 128 : (sc + 1) * 128],
                            rhs=wkvv_t[:, c * 512 : (c + 1) * 512],
                            start=(c == 0),
                            stop=(c == 5),
                        )
                    dst = vp_t[sc][:].rearrange("p (h e) -> p h e", e=65)
                    nc.vector.tensor_copy(
                        dst[:, :, 0:64], ps[:].rearrange("p (h d) -> p h d", d=64)
                    )
                    nc.vector.tensor_scalar_mul(
                        dst[:, :, 64:65],
                        ones8_t[:].rearrange("p (h o) -> p h o", o=1),
                        m01_t[:, sc : sc + 1],
                    )

                ops.append(v_proj)
            return ops

        def attn_units(gi):
            g0, gn = groups[gi]
            pairs = []
            p0 = g0
            while p0 < g0 + gn:
                np_ = min(2, g0 + gn - p0)
                pairs.append((p0, np_))
                p0 += np_
            return [(kc, p0, np_) for kc in range(4) for (p0, np_) in pairs]

        pend_pv = []  # [(kc, p0, np_, eE, eO)]
        n_done = [0] * 4  # PV s-chunks accumulated per kc
        pvq = [None] * 4  # current per-group psum pair for each kc

        def sc_group(sc):
            for g0, gn in groups:
                if g0 <= sc < g0 + gn:
                    return g0, gn
            raise AssertionError

        def emit_scores(kc, p0, np_):
            if V2_STAGE < 2:
                return
            # separate psum tiles per head: row-group-0 and row-group-64
            # outputs must not share a psum bank
            psE = psum.tile([128, 512], F32, tag="sc", bufs=2, name="psE")
            psO = psum.tile([128, 512], F32, tag="sc", bufs=2, name="psO")
            for i in range(np_):
                sc = p0 + i
                nc.tensor.matmul(
                    psE[:, i * T : (i + 1) * T],
                    lhsT=kt_t[kc][0:64, sc * 128 : (sc + 1) * 128],
                    rhs=qt_t[kc][0:64, :],
                    start=True,
                    stop=True,
                )
            for i in range(np_):
                sc = p0 + i
                nc.tensor.matmul(
                    psO[:, i * T : (i + 1) * T],
                    lhsT=kt_t[kc][64:128, sc * 128 : (sc + 1) * 128],
                    rhs=qt_t[kc][64:128, :],
                    start=True,
                    stop=True,
                )
            if V2_STAGE < 3:
                return
            eE = work.tile([128, 512], BF16, tag="e", bufs=6, name="eE")
            eO = work.tile([128, 512], BF16, tag="e", bufs=6, name="eO")
            nc.scalar.activation(
                eE[:, 0 : np_ * T], psE[:, 0 : np_ * T],
                mybir.ActivationFunctionType.Exp,
            )
            nc.scalar.activation(
                eO[:, 0 : np_ * T], psO[:, 0 : np_ * T],
                mybir.ActivationFunctionType.Exp,
            )
            if V2_STAGE < 4:
                return
            pend_pv.append((kc, p0, np_, eE, eO))

        def emit_pv():
            kc, p0, np_, eE, eO = pend_pv.pop(0)
            g0, gn = sc_group(p0)
            if p0 == g0:
                pvq[kc] = [
                    psum.tile([65, T], F32, tag="pv", bufs=4, name=f"pvq{hi}")
                    for hi in range(2)
                ]
            for i in range(np_):
                sc = p0 + i
                for hi, e in ((0, eE), (1, eO)):
                    h = 2 * kc + hi
                    nc.tensor.matmul(
                        pvq[kc][hi][:],
                        lhsT=vp_t[sc][:, h * 65 : h * 65 + 65],
                        rhs=e[:, i * T : (i + 1) * T],
                        start=(sc == g0),
                        stop=(sc == g0 + gn - 1),
                    )
            if p0 + np_ == g0 + gn:
                # fold the group's PV into the SBUF accumulator (DVE)
                for hi in range(2):
                    dst = pvacc_t[kc][:, hi * T : (hi + 1) * T]
                    if g0 == 0:
                        nc.vector.tensor_copy(dst, pvq[kc][hi][:])
                    else:
                        nc.vector.tensor_add(dst, dst, pvq[kc][hi][:])
            n_done[kc] += np_
            if n_done[kc] == n_sc and V2_STAGE >= 5:
                emit_norm(kc)

        def emit_norm(kc):
            # denominators live in pvacc row 64 (cols 0:256 headE, 256:512 headO).
            # Spread the 512 values across 128 partitions (t = p*4+j) so the
            # reciprocal uses 128 DVE lanes instead of 1, then gather back.
            nc.sync.dma_start(
                denp_t[:, kc * 4 : (kc + 1) * 4], pvacc_t[kc][64:65, :]
            )
            with nc.allow_low_precision(reason="softmax denom reciprocal in bf16"):
                nc.vector.reciprocal(
                    recb_t[:, kc * 4 : (kc + 1) * 4],
                    denp_t[:, kc * 4 : (kc + 1) * 4],
                )
            nc.gpsimd.dma_start(
                rechp_t[0:1, kc * 512 : (kc + 1) * 512],
                recb_t[:, kc * 4 : (kc + 1) * 4],
            )
            bc = psum.tile([128, 512], F32, tag="sc", bufs=2, name="bc")
            nc.tensor.matmul(
                bc[0:64, :],
                lhsT=ones64_t[:],
                rhs=rechp_t[0:1, kc * 512 : (kc + 1) * 512],
                start=True,
                stop=True,
            )
            nc.vector.tensor_mul(otE_t[kc][:], pvacc_t[kc][0:64, 0:T], bc[0:64, 0:T])
            nc.vector.tensor_mul(
                otO_t[kc][:], pvacc_t[kc][0:64, T : 2 * T], bc[0:64, T : 2 * T]
            )
            if V2_STAGE < 6:
                return
            # out projection contribution of this head pair (64-row chunks)
            for j, ot in ((2 * kc, otE_t[kc]), (2 * kc + 1, otO_t[kc])):
                for oi in range(4):
                    nc.tensor.matmul(
                        out_ps[oi // 2][:, (oi % 2) * T : (oi % 2 + 1) * T],
                        lhsT=wo_t[:, j * 512 + oi * 128 : j * 512 + (oi + 1) * 128],
                        rhs=ot[:],
                        start=(j == 0 and oi % 2 == 0),
                        stop=(j == 7 and oi % 2 == 1),
                    )

        out_ps = None
        for gi in range(G + 1):
            units = attn_units(gi - 1) if gi >= 1 else []
            pops = proj_ops(gi) if gi < G else []
            if gi == G:
                # allocate out-projection accumulators (w-ring is free now)
                out_ps = [
                    psum.tile([128, 512], F32, tag="w", bufs=2, name=f"ops{i}")
                    for i in range(2)
                ]
            if V2_STAGE < 2:
                units = []
            if not units:
                for op in pops:
                    op()
                continue
            # interleave proj ops between attention units
            if V2_NOILV:
                for op in pops:
                    op()
                pops = []
            k = 0
            for i, (kc, p0, np_) in enumerate(units):
                emit_scores(kc, p0, np_)
                while len(pend_pv) > 1:
                    emit_pv()
                k_to = (i + 1) * len(pops) // len(units) if pops else 0
                while k < k_to:
                    pops[k]()
                    k += 1
        while pend_pv:
            emit_pv()

        # ---- bias + store -------------------------------------------------
        for half in range(2):
            osb = work.tile([128, 512], F32, tag="osb", bufs=2, name="osb")
            if V2_STAGE >= 6:
                for eo2 in range(2):
                    eo = half * 2 + eo2
                    nc.vector.tensor_scalar_add(
                        osb[:, eo2 * T : (eo2 + 1) * T],
                        out_ps[half][:, eo2 * T : (eo2 + 1) * T],
                        bo_t[:, eo : eo + 1],
                    )
            else:
                nc.vector.memset(osb[:], 0.0)
            for eo2 in range(2):
                q = nc.sync if eo2 == 0 else nc.gpsimd
                q.dma_start(
                    outT_d[2 * half + eo2], osb[:, eo2 * T : (eo2 + 1) * T]
                )

    nc.compile()
    return nc


_NC = {}


def _get_nc(n_sc):
    if n_sc not in _NC:
        _NC[n_sc] = _build_program(n_sc)
    return _NC[n_sc]


def _prep_in_maps(x, context, key_padding_mask, Wq, Wkv, Wo, bo):
    keep = ~np.asarray(key_padding_mask)
    kept = keep.sum(axis=1)
    n_sc = max(1, -(-int(kept.max()) // 128))
    S_C = n_sc * 128

    scale = np.float32(D**-0.5)
    wqr = (
        (np.ascontiguousarray(Wq.T) * scale)
        .reshape(4, 128, 4 * 128)
        .transpose(1, 0, 2)
        .reshape(128, 4 * 512)
        .astype(NPBF16)
    )
    wkvT = np.ascontiguousarray(Wkv.T)  # [768, 1024]
    wkvk = (
        wkvT[:, :512].reshape(6, 128, 512).transpose(1, 0, 2).reshape(128, 6 * 512)
    ).astype(NPBF16)
    wkvv = (
        wkvT[:, 512:].reshape(6, 128, 512).transpose(1, 0, 2).reshape(128, 6 * 512)
    ).astype(NPBF16)
    wo64 = (
        np.ascontiguousarray(Wo.T)
        .reshape(8, 64, 512)
        .transpose(1, 0, 2)
        .reshape(64, 8 * 512)
    ).astype(NPBF16)
    bo_r = np.ascontiguousarray(bo.reshape(4, 128).T).astype(np.float32)

    in_maps = []
    for b in range(B):
        idx = np.nonzero(keep[b])[0]
        k_b = len(idx)
        ctxT = np.zeros((KV, S_C), np.float32)
        ctxT[:, :k_b] = context[b][idx].T
        ctxb = (
            ctxT.reshape(6, 128, S_C).transpose(1, 0, 2).reshape(128, 6 * S_C)
        ).astype(NPBF16)
        xr = (
            np.ascontiguousarray(x[b].T)
            .reshape(4, 128, T)
            .transpose(1, 0, 2)
            .reshape(128, 4 * T)
        ).astype(NPBF16)
        m01 = np.zeros((128, n_sc), np.float32)
        flat = np.arange(S_C) < k_b
        m01[:, :] = flat.reshape(n_sc, 128).T
        in_maps.append(
            dict(
                ctxb=ctxb,
                xr=xr,
                m01=m01,
                wqr=wqr,
                wkvk=wkvk,
                wkvv=wkvv,
                wo64=wo64,
                bo_r=bo_r,
            )
        )
    return in_maps, n_sc


def _run(inputs, trace=False, **kw):
    in_maps, n_sc = _prep_in_maps(**inputs)
    nc = _get_nc(n_sc)
    res = bass_utils.run_bass_kernel_spmd(
        nc, in_maps, core_ids=list(range(NC_CORES)), trace=trace, **kw
    )
    out = np.stack(
        [res.results[b]["outT"].reshape(E, T).T for b in range(B)]
    ).astype(np.float32)
    return out, res


def kernel(**inputs):
    out, _ = _run(inputs, trace=False)
    return out


if __name__ == "__main__":
    rng = np.random.default_rng(0)
    ins = dict(
        x=rng.standard_normal((B, T, E), dtype=np.float32),
        context=rng.standard_normal((B, S, KV), dtype=np.float32),
        key_padding_mask=rng.integers(0, 2, (B, S)).astype(bool),
        Wq=(rng.standard_normal((512, E), dtype=np.float32) * 0.02),
        Wkv=(rng.standard_normal((1024, KV), dtype=np.float32) * 0.02),
        Wo=(rng.standard_normal((E, 512), dtype=np.float32) * 0.02),
        bo=np.zeros(E, dtype=np.float32),
    )
    out = kernel(**ins)
    print("out", out.shape, out.dtype, np.abs(out).mean())
